# revision 29
# baseline (speedup 1.0000x reference)
"""GRU encoder + autoregressive decoder (seq2seq RNN) on 8 TRN2 cores.

Strategy: data-parallel over batch (512 -> 8 x 64), weights replicated.
Per core, the sequential recurrence runs locally:
  - h state kept BOTH in batch-major SBUF layout [64, 512] (for elementwise)
    and transposed hT [128, 4, 64] (stationary operand for matmuls).
  - Matmuls: out[batch, gate_cols] = hT.T @ WhhT with moving weights stored
    as float32r (FP22) -> 1 cycle/row at moving-free >= 256.
  - Biases folded into the matmuls via an appended ones-row on the
    ih-stationary (x / y) and a [1,512] bhh_n moving row.
  - sigmoid(x) computed as 0.5*tanh(0.5x)+0.5 so only the Tanh ACT table is
    used (avoids activation-table swap penalties).
  - h' = n*u + 0.5*(tz*h + h), u = 0.5 - 0.5*tz, tz = tanh(0.5*z_pre).
  - h' -> hT via 4 PE transposes per step.
"""

import numpy as np

N_CORES = 8
B = 64           # batch per core
T = 128          # encoder steps
I = 64           # input size
H = 512          # hidden size
O = 64           # output size
L = 300          # predict length
NK = H // 128    # K tiles over hidden dim
G = 512          # gate region width (cols per gate, = H)

_CACHE = {}
LAST_RESULTS = None


def _build_nc():
    from contextlib import ExitStack

    import concourse.bass as bass
    import concourse.tile as tile
    from concourse import bacc, mybir
    from concourse.masks import make_identity

    f32 = mybir.dt.float32
    f32r = mybir.dt.float32r
    AF = mybir.ActivationFunctionType
    ALU = mybir.AluOpType

    nc = bacc.Bacc(trn_type="TRN2")

    x_d = nc.dram_tensor("x", [T, I + 1, B], f32r, kind="ExternalInput")
    whh_d = nc.dram_tensor("whh", [H, 3 * H], f32r, kind="ExternalInput")
    wih_d = nc.dram_tensor("wih", [I + 1, 3 * H], f32r, kind="ExternalInput")
    bhhn_d = nc.dram_tensor("bhhn", [1, H], f32r, kind="ExternalInput")
    wo_d = nc.dram_tensor("wo", [H, O], f32r, kind="ExternalInput")
    bo_d = nc.dram_tensor("bo", [1, O], f32, kind="ExternalInput")
    out_d = nc.dram_tensor("out", [B, L * O], f32, kind="ExternalOutput")

    with tile.TileContext(nc) as tc, ExitStack() as ctx:
        singles = ctx.enter_context(tc.tile_pool(name="singles", bufs=1))
        xpool = ctx.enter_context(tc.tile_pool(name="xpool", bufs=3))
        gpsum = ctx.enter_context(tc.tile_pool(name="gpsum", bufs=1, space="PSUM"))
        tpsum = ctx.enter_context(tc.tile_pool(name="tpsum", bufs=2, space="PSUM"))
        ypsum = ctx.enter_context(tc.tile_pool(name="ypsum", bufs=1, space="PSUM"))

        dma = nc.default_dma_engine

        # --- weights / constants ---
        whh = singles.tile([128, NK, 3 * H], f32r)
        dma.dma_start(whh[:], whh_d[:].rearrange("(k p) j -> p k j", p=128))
        wih = singles.tile([I + 1, 3 * H], f32r)
        dma.dma_start(wih[:], wih_d[:])
        bhhn = singles.tile([1, H], f32r)
        dma.dma_start(bhhn[:], bhhn_d[:])
        wo = singles.tile([128, NK, O], f32r)
        dma.dma_start(wo[:], wo_d[:].rearrange("(k p) o -> p k o", p=128))
        bo_bc = singles.tile([B, O], f32)
        bo_ap = bo_d[:]
        dma.dma_start(
            bo_bc[:],
            bass.AP(tensor=bo_ap.tensor, offset=bo_ap.offset,
                    ap=[[0, B], list(bo_ap.ap[-1])]),
        )

        ones_f = singles.tile([1, B], f32)
        nc.vector.memset(ones_f[:], 1.0)
        ones_s = singles.tile([1, B], f32r)
        nc.scalar.activation(ones_s[:], ones_f[:], AF.Copy)
        half_b = singles.tile([B, 1], f32)
        nc.vector.memset(half_b[:], 0.5)
        zero_b = singles.tile([B, 1], f32)
        nc.vector.memset(zero_b[:], 0.0)
        ident = singles.tile([B, B], f32)
        make_identity(nc, ident[:])

        # --- state ---
        h_sb = singles.tile([B, H], f32)
        nc.vector.memset(h_sb[:], 0.0)
        hz = singles.tile([128, NK, B], f32)
        nc.vector.memset(hz[:], 0.0)
        hT = singles.tile([128, NK, B], f32r)
        nc.scalar.activation(hT[:], hz[:], AF.Copy)
        yz = singles.tile([O + 1, B], f32)
        nc.vector.memset(yz[:], 0.0)
        nc.vector.memset(yz[O:O + 1, :], 1.0)
        yaug = singles.tile([O + 1, B], f32r)
        nc.scalar.activation(yaug[:], yz[:], AF.Copy)
        out_buf = singles.tile([B, L * O], f32)

        # --- elementwise temporaries (persistent, reused each step) ---
        tz = singles.tile([B, H], f32)
        u = singles.tile([B, H], f32)
        w1 = singles.tile([B, H], f32)
        s = singles.tile([B, H], f32)
        sh = singles.tile([B, H], f32)
        hnh = singles.tile([B, H], f32)
        q = singles.tile([B, H], f32)
        tr = singles.tile([B, H], f32)
        m1 = singles.tile([B, H], f32)
        arg = singles.tile([B, H], f32)
        nn = singles.tile([B, H], f32)
        nu = singles.tile([B, H], f32)

        def gru_step(x_stat):
            """One GRU step: h (h_sb/hT) updated in place.
            x_stat: [I+1, B] f32r stationary (input with ones row)."""
            g = gpsum.tile([B, 4, G], f32)  # regions: 0=r_pre 1=z_pre 2=inn 3=hn
            # hh matmuls, grouped by stationary hT k-tile
            for k in range(NK):
                st = k == 0
                nc.tensor.matmul(g[:, 1, :], hT[:, k, :], whh[:, k, G:2 * G],
                                 start=st, stop=False)
                nc.tensor.matmul(g[:, 3, :], hT[:, k, :], whh[:, k, 2 * G:3 * G],
                                 start=st, stop=False)
                nc.tensor.matmul(g[:, 0, :], hT[:, k, :], whh[:, k, 0:G],
                                 start=st, stop=False)
            # hn += bhh_n (ones-row stationary)
            nc.tensor.matmul(g[:, 3, :], ones_s[:], bhhn[:], start=False, stop=True)
            # ih matmuls (bias rows folded into wih row 64)
            nc.tensor.matmul(g[:, 1, :], x_stat, wih[:, G:2 * G], start=False, stop=True)
            nc.tensor.matmul(g[:, 2, :], x_stat, wih[:, 2 * G:3 * G], start=True, stop=True)
            nc.tensor.matmul(g[:, 0, :], x_stat, wih[:, 0:G], start=False, stop=True)

            # z-route: z = 0.5*tanh(0.5*z_pre) + 0.5
            nc.scalar.activation(tz[:], g[:, 1, :], AF.Tanh, bias=zero_b[:], scale=0.5)
            nc.scalar.activation(u[:], tz[:], AF.Identity, bias=half_b[:], scale=-0.5)
            nc.vector.tensor_mul(w1[:], tz[:], h_sb[:])
            nc.vector.tensor_add(s[:], w1[:], h_sb[:])
            nc.scalar.activation(sh[:], s[:], AF.Identity, bias=zero_b[:], scale=0.5)
            # q = inn + 0.5*hn   (hn includes bhh_n)
            nc.scalar.activation(hnh[:], g[:, 3, :], AF.Identity, bias=zero_b[:],
                                 scale=0.5)
            nc.vector.tensor_add(q[:], g[:, 2, :], hnh[:])
            # r-route: n = tanh(q + tr*(0.5*hn)), tr = tanh(0.5*r_pre)
            nc.scalar.activation(tr[:], g[:, 0, :], AF.Tanh, bias=zero_b[:], scale=0.5)
            nc.vector.tensor_mul(m1[:], tr[:], hnh[:])
            nc.vector.tensor_add(arg[:], m1[:], q[:])
            nc.scalar.activation(nn[:], arg[:], AF.Tanh, bias=zero_b[:])
            nc.vector.tensor_mul(nu[:], nn[:], u[:])
            # h' = n*u + 0.5*(w1 + h) -- in place
            nc.vector.tensor_add(h_sb[:], nu[:], sh[:])
            # transpose h' -> hT
            tp = tpsum.tile([128, NK, B], f32)
            for c in range(NK):
                nc.tensor.transpose(tp[:, c, :], h_sb[:, c * 128:(c + 1) * 128],
                                    ident[:])
                nc.scalar.activation(hT[:, c, :], tp[:, c, :], AF.Copy)

        # persistent PSUM scratch: cols 0:B = decoder y matmul out [B, O],
        # cols B:2B = y transpose out [O, B].
        ydual = ypsum.tile([B, 2 * B], f32)

        # ---------------- encoder ----------------
        for t in range(T):
            x_t = xpool.tile([I + 1, B], f32r)
            dma.dma_start(x_t[:], x_d[t])
            gru_step(x_t[:])

        # ---------------- decoder ----------------
        for t in range(L):
            yp = ydual[:, 0:B]
            for k in range(NK):
                nc.tensor.matmul(yp, hT[:, k, :], wo[:, k, :],
                                 start=(k == 0), stop=(k == NK - 1))
            ysl = out_buf[:, t * O:(t + 1) * O]
            nc.vector.tensor_add(ysl, yp, bo_bc[:])
            if t < L - 1:
                ytp = ydual[:, B:2 * B]
                nc.tensor.transpose(ytp, ysl, ident[:])
                nc.scalar.activation(yaug[0:O, :], ytp, AF.Copy)
                gru_step(yaug[:])

        dma.dma_start(out_d[:], out_buf[:])

    nc.finalize()
    return nc


def _prep_in_maps(input_, Wih, Whh, bih, bhh, Wo, bo):
    f32 = np.float32
    input_ = np.asarray(input_, f32)
    Wih = np.asarray(Wih, f32)
    Whh = np.asarray(Whh, f32)
    bih = np.asarray(bih, f32)
    bhh = np.asarray(bhh, f32)
    Wo = np.asarray(Wo, f32)
    bo = np.asarray(bo, f32)

    whhT = np.ascontiguousarray(Whh.T)                    # [H, 3H]
    wih_aug = np.empty((I + 1, 3 * H), f32)
    wih_aug[:I] = Wih.T                                   # [I, 3H]
    bias_row = bih.copy()
    bias_row[: 2 * H] += bhh[: 2 * H]                     # r,z get bih+bhh
    wih_aug[I] = bias_row                                 # n gets bih only
    bhhn = np.ascontiguousarray(bhh[2 * H:].reshape(1, H))
    woT = np.ascontiguousarray(Wo.T)                      # [H, O]
    bo_r = np.ascontiguousarray(bo.reshape(1, O))

    shared = {"whh": whhT, "wih": wih_aug, "bhhn": bhhn, "wo": woT, "bo": bo_r}
    in_maps = []
    for c in range(N_CORES):
        xc = input_[c * B:(c + 1) * B]                    # [B, T, I]
        xaug = np.empty((T, I + 1, B), f32)
        xaug[:, :I, :] = xc.transpose(1, 2, 0)
        xaug[:, I, :] = 1.0
        in_maps.append({"x": np.ascontiguousarray(xaug), **shared})
    return in_maps


def kernel(**inputs):
    global LAST_RESULTS
    from concourse.bass_utils import run_bass_kernel_spmd

    pl = int(np.asarray(inputs["predict_length"]))
    assert pl == L, f"kernel compiled for predict_length={L}, got {pl}"

    in_maps = _prep_in_maps(
        inputs["input"], inputs["Wih"], inputs["Whh"],
        inputs["bih"], inputs["bhh"], inputs["Wo"], inputs["bo"],
    )

    if "nc" not in _CACHE:
        _CACHE["nc"] = _build_nc()
    nc = _CACHE["nc"]

    res = run_bass_kernel_spmd(nc, in_maps, core_ids=list(range(N_CORES)))
    LAST_RESULTS = res

    out = np.empty((N_CORES * B, L, O), np.float32)
    for c in range(N_CORES):
        out[c * B:(c + 1) * B] = res.results[c]["out"].reshape(B, L, O)
    return out
